# revision 1
# baseline (speedup 1.0000x reference)
"""Trainium2 Bass kernel for the dense real-space long-range kernel
(N=6144 atoms, B=8 periodic cells, screened-Coulomb pair energy with
minimum-image convention, row-summed per atom).

Strategy: batch is sorted, and cross-graph pairs are masked out by the
reference, so the N x N problem is block-diagonal over the 8 graphs.
One graph per NeuronCore.  All math is done in fractional coordinates:

  f_k[i,j]  = frac_k[j] - frac_k[i]          (DVE tensor_scalar, broadcast row)
  r_k       = round(f_k)                     (DVE magic-number round)
  y         = (f - r) @ C                    (2 accumulating fp32r matmuls,
                                              block-diag cell stationary)
  q         = sum_j y_j^2                    (ACT Square + ones-blockdiag matmul)
  kern      = exp(-sigma*sqrt(q+soft^2)) / sqrt(q+soft^2)
                                             (ACT Sqrt/Exp + DVE recip approx)
  acc[j]    = sum_i src_i * kern[i,j]        (fp32r matvec, PSUM accumulate
                                              over row blocks; row sum == col
                                              sum by symmetry)
  E[j]      = 0.5*src_j*acc_j - 0.5*src_j^2*exp(-sigma*soft)/soft

Atoms are processed in groups of 32 (3 coordinate rows per atom = 96
partitions); 4 groups form a 128-atom macro block whose q tile uses the
full partition width for the transcendental tail.  The 32-atom group size
matches the PE tile_position granularity for the stage-2 row offsets.
"""
import numpy as np

GA = 42            # atoms per k-interleaved row group
ROWS = 3 * GA      # 126 partitions per group tile
GPM = 3            # groups per macro block
MACRO = GA * GPM   # 126 atoms per macro
MAGIC = 12582912.0  # 1.5 * 2**23: (x + MAGIC) - MAGIC == round(x) for |x| < 2**22
NCORES = 8
CHUNK = 512        # PSUM bank / fp32 matmul free-dim limit

_cache = {}


def _build(n_macros, cols, sigma, soft):
    import concourse.bacc as bacc
    import concourse.mybir as mybir
    import concourse.tile as tile

    f32 = mybir.dt.float32
    f32r = mybir.dt.float32r
    alu = mybir.AluOpType
    act = mybir.ActivationFunctionType

    n_groups = GPM * n_macros
    pw = -(-cols // CHUNK) * CHUNK
    chunks = [(c, min(cols, c + CHUNK)) for c in range(0, cols, CHUNK)]
    soft2 = float(np.float32(soft) * np.float32(soft))

    nc = bacc.Bacc("TRN2", target_bir_lowering=False, debug=False)
    # const AP for the Sqrt bias (soft^2), registered like the built-ins
    t = nc.alloc_sbuf_tensor("const-soft2", [128, 1], f32)
    nc.gpsimd.memset(t.ap(), soft2)
    nc.const_aps.aps[(f32, soft2)] = t.ap()
    nc.all_engine_barrier()

    FB = nc.declare_dram_parameter("FB", [ROWS, cols], f32, isOutput=False)
    NEGFA = nc.declare_dram_parameter("NEGFA", [ROWS, n_groups], f32, isOutput=False)
    CB = nc.declare_dram_parameter("CB", [ROWS, ROWS], f32r, isOutput=False)
    CBN = nc.declare_dram_parameter("CBN", [ROWS, ROWS], f32r, isOutput=False)
    ONESB = nc.declare_dram_parameter("ONESB", [ROWS, GPM * MACRO], f32r, isOutput=False)
    SRCST = nc.declare_dram_parameter("SRCST", [MACRO, n_macros], f32r, isOutput=False)
    A1 = nc.declare_dram_parameter("A1", [1, cols], f32, isOutput=False)
    A2 = nc.declare_dram_parameter("A2", [1, cols], f32, isOutput=False)
    OUT = nc.declare_dram_parameter("OUT", [1, cols], f32, isOutput=True)

    with tile.TileContext(nc) as tc:
        with tc.tile_pool(name="const", bufs=1) as cpool, \
             tc.tile_pool(name="work", bufs=3) as pool, \
             tc.tile_pool(name="ypsum", bufs=2, space="PSUM") as ypool, \
             tc.tile_pool(name="qpsum", bufs=1, space="PSUM") as qpool, \
             tc.tile_pool(name="apsum", bufs=1, space="PSUM") as apool:
            fb = cpool.tile([ROWS, cols], f32)
            negfa = cpool.tile([ROWS, n_groups], f32)
            cb = cpool.tile([ROWS, ROWS], f32r)
            cbn = cpool.tile([ROWS, ROWS], f32r)
            onesb = cpool.tile([ROWS, GPM * MACRO], f32r)
            srcst = cpool.tile([MACRO, n_macros], f32r)
            a1 = cpool.tile([1, cols], f32)
            a2 = cpool.tile([1, cols], f32)
            nc.sync.dma_start(fb[:], FB[:])
            nc.sync.dma_start(negfa[:], NEGFA[:])
            nc.sync.dma_start(cb[:], CB[:])
            nc.sync.dma_start(cbn[:], CBN[:])
            nc.sync.dma_start(onesb[:], ONESB[:])
            nc.sync.dma_start(srcst[:], SRCST[:])
            nc.sync.dma_start(a1[:], A1[:])
            nc.sync.dma_start(a2[:], A2[:])

            acc = apool.tile([1, pw], f32)

            for m in range(n_macros):
                q = qpool.tile([MACRO, pw], f32, tag="q")
                for tgi in range(GPM):
                    g = GPM * m + tgi
                    f = pool.tile([ROWS, cols], f32r, tag="f")
                    nc.vector.tensor_scalar(f[:], fb[:], negfa[:, g:g + 1], None,
                                            alu.add)
                    r = pool.tile([ROWS, cols], f32r, tag="r")
                    nc.vector.tensor_scalar(r[:], f[:], MAGIC, MAGIC,
                                            alu.add, alu.subtract)
                    y = ypool.tile([ROWS, pw], f32, tag="y")
                    for (c0, c1) in chunks:
                        nc.tensor.matmul(y[:, c0:c1], cb[:], f[:, c0:c1],
                                         start=True, stop=False)
                        nc.tensor.matmul(y[:, c0:c1], cbn[:], r[:, c0:c1],
                                         start=False, stop=True)
                    sq = pool.tile([ROWS, cols], f32r, tag="sq")
                    nc.scalar.activation(sq[:], y[:, 0:cols], act.Square)
                    # ones-blockdiag zero-padded to map group tgi's atoms to
                    # q rows 32*tgi..32*tgi+31; accumulate all 4 groups
                    ob = onesb[:, MACRO * tgi:MACRO * (tgi + 1)]
                    for (c0, c1) in chunks:
                        nc.tensor.matmul(q[:, c0:c1], ob, sq[:, c0:c1],
                                         start=(tgi == 0), stop=(tgi == GPM - 1))
                rt = pool.tile([MACRO, cols], f32, tag="rt")
                nc.scalar.activation(rt[:], q[:, 0:cols], act.Sqrt, bias=soft2)
                et = pool.tile([MACRO, cols], f32, tag="et")
                nc.scalar.activation(et[:], rt[:], act.Exp, scale=-sigma)
                rcp = pool.tile([MACRO, cols], f32, tag="rcp")
                nc.vector.reciprocal_approx_fast(out=rcp[:], in_=rt[:])
                kern = pool.tile([MACRO, cols], f32r, tag="kern")
                nc.vector.tensor_tensor(kern[:], et[:], rcp[:], alu.mult)
                for (c0, c1) in chunks:
                    nc.tensor.matmul(acc[0:1, c0:c1], srcst[:, m:m + 1],
                                     kern[:, c0:c1],
                                     start=(m == 0), stop=(m == n_macros - 1))

            t1 = pool.tile([1, cols], f32, tag="t1")
            nc.vector.tensor_tensor(t1[:], acc[0:1, 0:cols], a1[:], alu.mult)
            eo = pool.tile([1, cols], f32, tag="eo")
            nc.vector.tensor_tensor(eo[:], t1[:], a2[:], alu.subtract)
            nc.sync.dma_start(OUT[:], eo[:])
    nc.compile()
    return nc


def _get_program(n_macros, cols, sigma, soft):
    key = (n_macros, cols, round(sigma, 9), round(soft, 9))
    if key not in _cache:
        _cache[key] = _build(n_macros, cols, sigma, soft)
    return _cache[key]


LAST_EXEC_TIME_NS = None


def kernel(pos, batch, cell, source, screening, softening, *, _trace=False):
    global LAST_EXEC_TIME_NS
    from concourse.bass_utils import run_bass_kernel_spmd

    pos = np.asarray(pos)
    batch = np.asarray(batch)
    cell = np.asarray(cell)
    source = np.asarray(source, dtype=np.float32)
    sigma = float(np.asarray(screening, dtype=np.float32))
    soft = float(np.asarray(softening, dtype=np.float32))

    n = pos.shape[0]
    nb = cell.shape[0]
    bi = batch.astype(np.int64)
    counts = np.bincount(bi, minlength=nb)
    starts = np.concatenate([[0], np.cumsum(counts)])
    assert nb == NCORES and np.all(np.diff(bi) >= 0)

    # host precompute in float64
    inv = np.linalg.inv(cell.astype(np.float64))
    frac = np.empty((n, 3), dtype=np.float64)
    for g in range(nb):
        i0, i1 = starts[g], starts[g + 1]
        frac[i0:i1] = pos[i0:i1].astype(np.float64) @ inv[g]
    frac32 = frac.astype(np.float32)

    namax = int(counts.max())
    n_macros = -(-namax // MACRO)
    cols = MACRO * n_macros       # padded atom count per core
    n_groups = GPM * n_macros
    diag_c = float(np.exp(-np.float64(sigma) * np.float64(soft)) / np.float64(soft))

    idx_atom = np.arange(ROWS) // 3
    idx_k = np.arange(ROWS) % 3

    in_maps = []
    for g in range(nb):
        i0, i1 = starts[g], starts[g + 1]
        ng = i1 - i0
        fpad = np.zeros((cols, 3), dtype=np.float32)
        fpad[:ng] = frac32[i0:i1]
        spad = np.zeros(cols, dtype=np.float32)
        spad[:ng] = source[i0:i1]

        fb = np.ascontiguousarray(np.tile(fpad.T, (GA, 1)))  # [96, cols]: row p -> coord p%3
        negfa = np.zeros((ROWS, n_groups), dtype=np.float32)
        for t in range(n_groups):
            a = t * GA + idx_atom
            negfa[:, t] = -fpad[a, idx_k]
        C = cell[g].astype(np.float32)
        cbm = np.zeros((ROWS, ROWS), dtype=np.float32)
        for i in range(GA):
            cbm[3 * i:3 * i + 3, 3 * i:3 * i + 3] = C
        onesb = np.zeros((ROWS, GPM, MACRO), dtype=np.float32)
        for t in range(GPM):
            for i in range(GA):
                onesb[3 * i:3 * i + 3, t, GA * t + i] = 1.0
        onesb = np.ascontiguousarray(onesb.reshape(ROWS, GPM * MACRO))
        srcst = np.zeros((MACRO, n_macros), dtype=np.float32)
        for m in range(n_macros):
            srcst[:, m] = spad[m * MACRO: m * MACRO + MACRO]
        a1 = (0.5 * spad)[None, :].astype(np.float32)
        a2 = (0.5 * spad.astype(np.float64) ** 2 * diag_c)[None, :].astype(np.float32)
        in_maps.append({
            "FB": fb, "NEGFA": negfa, "CB": cbm, "CBN": -cbm,
            "ONESB": onesb, "SRCST": srcst, "A1": a1, "A2": a2,
        })

    nc = _get_program(n_macros, cols, sigma, soft)
    res = run_bass_kernel_spmd(nc, in_maps, list(range(NCORES)), trace=_trace)
    LAST_EXEC_TIME_NS = res.exec_time_ns

    out = np.zeros((n, 1), dtype=np.float32)
    for g in range(nb):
        i0, i1 = starts[g], starts[g + 1]
        out[i0:i1, 0] = res.results[g]["OUT"][0, : i1 - i0]
    return out



# revision 3
# speedup vs baseline: 1.0899x; 1.0899x over previous
"""Trainium2 Bass kernel: dense screened-Coulomb pair energy with periodic
minimum-image convention (N=6144 atoms, B=8 cells), row-summed per atom.

batch is sorted and cross-graph pairs are masked, so the N x N problem is
block-diagonal over the 8 graphs: one graph per NeuronCore.

Math (fractional coords, fs = frac - 0.5):
  f_k[i,j] = fs_k[j] - fs_k[i]            DVE tensor_scalar fp16 (4x mode)
  r_k      = round(f_k)                   DVE magic-number round fp16 (4x)
  y        = (f - r) @ C                  PE: 2 accumulating fp16 matmuls
                                          (block-diag cell stationary)
  sq       = y^2                          ACT Square / custom DVE op (split)
  q        = sum_k y_k^2                  PE: ones-blockdiag fp16 matmul
  kern     = exp(-sigma*r)/r, r=sqrt(q+soft^2)
           = exp(-0.5*t - sigma*exp(0.5*t + ln(sigma)))  with t = ln(q+soft^2)
                                          ACT Ln, ACT Exp, DVE stt, ACT Exp
                                          -- Ln/Exp/Square share ONE table set
                                          (natural_log_exp_and_others): no
                                          ACT_TABLE_LOAD thrash, no reciprocal.
  acc[j]   = sum_i src_i * kern[i,j]      PE matvec, PSUM-resident accumulator
                                          (row sum == col sum by symmetry)
  host: E[j] = 0.5*src_j*acc_j - 0.5*src_j^2*exp(-sigma*soft)/soft

Atoms in groups of GA=42 (3 coord rows per atom = 126 partitions); 3 groups
form a 126-atom macro whose q/kern tiles use the full partition width.
"""
import numpy as np

GA = 42            # atoms per row group
ROWS = 3 * GA      # 126 partitions per group tile
GPM = 3            # groups per macro block
MACRO = GA * GPM   # 126 atoms per macro
MAGIC = 12582912.0  # 1.5 * 2**23: (x + MAGIC) - MAGIC == round(x) for |x| < 2**22
NCORES = 8
CHUNK = 512        # PSUM bank limit for one matmul output (fp32 values)

_cache = {}
_pair_sq = None


def _get_pair_sq():
    """Register (once) a single-source custom DVE op computing out = in0^2.
    Used for the DVE share of the y->sq squares (PSUM source, so stock
    tensor_tensor(y, y) is illegal: both streams would be PSUM)."""
    global _pair_sq
    if _pair_sq is None:
        import concourse.dve_ops as dve_ops
        from concourse.dve_spec import Spec, Src0, sq, lower, _has_src1
        from concourse.dve_uop import DveOpSpec

        spec = Spec(body=sq(Src0),
                    reference=lambda in0, in1, s0, s1, imm2: in0 * in0)
        opcode = dve_ops._CUSTOM_DVE_ROW_BASE + len(dve_ops.OPS)
        shas = {}
        for ver in ("v3", "v4"):
            tmp = DveOpSpec(name="PAIR_SQ", opcode=opcode,
                            uops=lower(spec, ver=ver), rd1_en=_has_src1(spec))
            shas[ver] = tmp.sha(ver)
        op = dve_ops.DveOp("PAIR_SQ", spec, subdim=False, uops_sha=shas)
        dve_ops.OPS.append(op)
        dve_ops.CUSTOM_DVE_SPECS[op.name] = op.spec
        dve_ops._SUB_OPCODE_FOR_NAME[op.name] = opcode
        _pair_sq = op
    return _pair_sq


def _build(n_macros, n_groups, cols, sigma, soft):
    import concourse.bacc as bacc
    import concourse.mybir as mybir
    import concourse.tile as tile

    f32 = mybir.dt.float32
    f16 = mybir.dt.float16
    alu = mybir.AluOpType
    act = mybir.ActivationFunctionType
    pair_sq = _get_pair_sq()

    soft2 = float(np.float32(soft) * np.float32(soft))
    lnsig = float(np.log(np.float64(sigma)))
    chunks = [(c, min(cols, c + CHUNK)) for c in range(0, cols, CHUNK)]
    pw = -(-cols // CHUNK) * CHUNK

    nc = bacc.Bacc("TRN2", target_bir_lowering=False, debug=False)
    # const APs for activation biases, registered like the built-ins
    for name, val in (("c-soft2", soft2), ("c-lnsig", lnsig)):
        t = nc.alloc_sbuf_tensor(name, [128, 1], f32)
        nc.gpsimd.memset(t.ap(), val)
        nc.const_aps.aps[(f32, val)] = t.ap()
    nc.all_engine_barrier()

    FB = nc.declare_dram_parameter("FB", [ROWS, cols], f16, isOutput=False)
    NEGFA = nc.declare_dram_parameter("NEGFA", [ROWS, n_groups], f32, isOutput=False)
    CB = nc.declare_dram_parameter("CB", [ROWS, ROWS], f16, isOutput=False)
    CBN = nc.declare_dram_parameter("CBN", [ROWS, ROWS], f16, isOutput=False)
    ONESB = nc.declare_dram_parameter("ONESB", [ROWS, GPM * MACRO], f16, isOutput=False)
    SRCST = nc.declare_dram_parameter("SRCST", [MACRO, n_macros], f16, isOutput=False)
    OUT = nc.declare_dram_parameter("OUT", [1, cols], f32, isOutput=True)

    # group g -> (macro, slot); last macro may have < GPM groups
    def macro_groups(m):
        return [g for g in range(GPM * m, min(GPM * (m + 1), n_groups))]

    with tile.TileContext(nc) as tc:
        with tc.tile_pool(name="const", bufs=1) as cpool, \
             tc.tile_pool(name="work", bufs=3) as pool, \
             tc.tile_pool(name="ypsum", bufs=2, space="PSUM") as ypool, \
             tc.tile_pool(name="qpsum", bufs=1, space="PSUM") as qpool, \
             tc.tile_pool(name="apsum", bufs=1, space="PSUM") as apool:
            fb = cpool.tile([ROWS, cols], f16)
            negfa = cpool.tile([ROWS, n_groups], f32)
            cb = cpool.tile([ROWS, ROWS], f16)
            cbn = cpool.tile([ROWS, ROWS], f16)
            onesb = cpool.tile([ROWS, GPM * MACRO], f16)
            srcst = cpool.tile([MACRO, n_macros], f16)
            nc.sync.dma_start(fb[:], FB[:])
            nc.sync.dma_start(negfa[:], NEGFA[:])
            nc.sync.dma_start(cb[:], CB[:])
            nc.sync.dma_start(cbn[:], CBN[:])
            nc.sync.dma_start(onesb[:], ONESB[:])
            nc.sync.dma_start(srcst[:], SRCST[:])

            acc = apool.tile([1, pw], f32)

            sq_on_act = 0  # alternate the y->sq square between ACT and DVE
            for m in range(n_macros):
                groups = macro_groups(m)
                # rows of q/kern actually written by this macro's groups
                mrows = GA * len(groups)
                q = qpool.tile([MACRO, pw], f32, tag="q")
                for ti, g in enumerate(groups):
                    f = pool.tile([ROWS, cols], f16, tag="f")
                    nc.vector.tensor_scalar(f[:], fb[:], negfa[:, g:g + 1],
                                            None, alu.add)
                    r = pool.tile([ROWS, cols], f16, tag="r")
                    nc.vector.tensor_scalar(r[:], f[:], MAGIC, MAGIC,
                                            alu.add, alu.subtract)
                    y = ypool.tile([ROWS, pw], f32, tag="y")
                    for (c0, c1) in chunks:
                        nc.tensor.matmul(y[:, c0:c1], cb[:], f[:, c0:c1],
                                         start=True, stop=False)
                    for (c0, c1) in chunks:
                        nc.tensor.matmul(y[:, c0:c1], cbn[:], r[:, c0:c1],
                                         start=False, stop=True)
                    sq = pool.tile([ROWS, cols], f16, tag="sq")
                    if sq_on_act:
                        nc.scalar.activation(sq[:], y[:, 0:cols], act.Square)
                    else:
                        nc.vector._custom_dve(pair_sq, out=sq[:],
                                              in0=y[:, 0:cols])
                    sq_on_act ^= 1
                    ob = onesb[:, MACRO * ti:MACRO * (ti + 1)]
                    for (c0, c1) in chunks:
                        nc.tensor.matmul(q[:, c0:c1], ob, sq[:, c0:c1],
                                         start=(ti == 0),
                                         stop=(ti == len(groups) - 1))
                # tail: kern = exp(-0.5*t - sigma*exp(0.5*t + ln(sigma)))
                t = pool.tile([MACRO, cols], f16, tag="t")
                nc.scalar.activation(t[0:mrows], q[0:mrows, 0:cols], act.Ln,
                                     bias=soft2)
                rtp = pool.tile([MACRO, cols], f16, tag="rtp")
                nc.scalar.activation(rtp[0:mrows], t[0:mrows], act.Exp,
                                     bias=lnsig, scale=0.5)
                w = pool.tile([MACRO, cols], f16, tag="w")
                nc.vector.scalar_tensor_tensor(w[0:mrows], t[0:mrows], -0.5,
                                               rtp[0:mrows],
                                               alu.mult, alu.subtract)
                kern = pool.tile([MACRO, cols], f16, tag="kern")
                nc.scalar.activation(kern[0:mrows], w[0:mrows], act.Exp)
                for (c0, c1) in chunks:
                    nc.tensor.matmul(acc[0:1, c0:c1], srcst[0:mrows, m:m + 1],
                                     kern[0:mrows, c0:c1],
                                     start=(m == 0), stop=(m == n_macros - 1))

            eo = pool.tile([1, cols], f32, tag="eo")
            nc.scalar.copy(eo[:], acc[0:1, 0:cols])
            nc.sync.dma_start(OUT[:], eo[:])
    nc.compile()
    return nc


def _get_program(n_macros, n_groups, cols, sigma, soft):
    key = (n_macros, n_groups, cols, round(sigma, 9), round(soft, 9))
    if key not in _cache:
        _cache[key] = _build(n_macros, n_groups, cols, sigma, soft)
    return _cache[key]


LAST_EXEC_TIME_NS = None


def kernel(pos, batch, cell, source, screening, softening, *, _trace=False):
    global LAST_EXEC_TIME_NS
    from concourse.bass_utils import run_bass_kernel_spmd

    pos = np.asarray(pos)
    cell = np.asarray(cell)
    source = np.asarray(source, dtype=np.float32)
    sigma = float(np.asarray(screening, dtype=np.float32))
    soft = float(np.asarray(softening, dtype=np.float32))

    n = pos.shape[0]
    nb = cell.shape[0]
    bi = np.asarray(batch).astype(np.int64)
    counts = np.bincount(bi, minlength=nb)
    starts = np.concatenate([[0], np.cumsum(counts)])
    assert nb == NCORES and np.all(np.diff(bi) >= 0)

    # host precompute in float64; fs = frac - 0.5 halves the fp16 repr error
    inv = np.linalg.inv(cell.astype(np.float64))
    fs = np.empty((n, 3), dtype=np.float64)
    for g in range(nb):
        i0, i1 = starts[g], starts[g + 1]
        fs[i0:i1] = pos[i0:i1].astype(np.float64) @ inv[g] - 0.5
    fs16 = fs.astype(np.float16)

    namax = int(counts.max())
    cols = -(-namax // 8) * 8          # padded atom count per core
    n_groups = -(-namax // GA)
    n_macros = -(-namax // MACRO)
    diag_c = float(np.exp(-np.float64(sigma) * np.float64(soft)) / np.float64(soft))

    idx_atom = np.arange(ROWS) // 3
    idx_k = np.arange(ROWS) % 3

    in_maps = []
    for g in range(nb):
        i0, i1 = starts[g], starts[g + 1]
        ng = i1 - i0
        fpad = np.zeros((cols, 3), dtype=np.float16)
        fpad[:ng] = fs16[i0:i1]

        fb = np.ascontiguousarray(np.tile(fpad.T, (GA, 1)))  # [126, cols]
        negfa = np.zeros((ROWS, n_groups), dtype=np.float32)
        for t in range(n_groups):
            a = np.minimum(t * GA + idx_atom, cols - 1)
            negfa[:, t] = -fpad[a, idx_k].astype(np.float32)
        C = cell[g].astype(np.float16)
        cbm = np.zeros((ROWS, ROWS), dtype=np.float16)
        for i in range(GA):
            cbm[3 * i:3 * i + 3, 3 * i:3 * i + 3] = C
        onesb = np.zeros((ROWS, GPM, MACRO), dtype=np.float16)
        for t in range(GPM):
            for i in range(GA):
                onesb[3 * i:3 * i + 3, t, GA * t + i] = 1.0
        onesb = np.ascontiguousarray(onesb.reshape(ROWS, GPM * MACRO))
        spad = np.zeros(MACRO * n_macros, dtype=np.float16)
        spad[:ng] = source[i0:i1].astype(np.float16)
        srcst = np.ascontiguousarray(
            spad.reshape(n_macros, MACRO).T)       # [MACRO, n_macros]
        in_maps.append({
            "FB": fb, "NEGFA": negfa, "CB": cbm, "CBN": -cbm,
            "ONESB": onesb, "SRCST": srcst,
        })

    nc = _get_program(n_macros, n_groups, cols, sigma, soft)
    res = run_bass_kernel_spmd(nc, in_maps, list(range(NCORES)), trace=_trace)
    LAST_EXEC_TIME_NS = res.exec_time_ns

    out = np.zeros((n, 1), dtype=np.float32)
    for g in range(nb):
        i0, i1 = starts[g], starts[g + 1]
        ng = i1 - i0
        accg = res.results[g]["OUT"][0, :ng].astype(np.float64)
        s = source[i0:i1].astype(np.float64)
        out[i0:i1, 0] = (0.5 * s * accg - 0.5 * s * s * diag_c).astype(np.float32)
    return out


# revision 4
# speedup vs baseline: 1.3695x; 1.2565x over previous
"""Trainium2 Bass kernel: dense screened-Coulomb pair energy with periodic
minimum-image convention (N=6144 atoms, B=8 cells), row-summed per atom.

batch is sorted and cross-graph pairs are masked, so the N x N problem is
block-diagonal over the 8 graphs: one graph per NeuronCore.

Math (fractional coords, fs = frac - 0.5):
  f_k[i,j] = fs_k[j] - fs_k[i]            DVE tensor_scalar fp16
  r_k      = round(f_k)                   DVE magic-number round fp16
  y        = (f - r) @ C                  PE: 2 accumulating fp16 matmuls
                                          (block-diag cell stationary, 128-wide
                                          weights for fast-weight-load)
  sq       = y^2                          ACT Square / custom DVE op (split)
  q        = sum_k y_k^2                  PE: ones-blockdiag fp16 matmul
  kern     = exp(-sigma*r)/r, r=sqrt(q+soft^2)
           = exp(-0.5*t - sigma*exp(0.5*t + ln(sigma)))  with t = ln(q+soft^2)
                                          ACT Ln, ACT Exp, DVE stt, ACT Exp
                                          -- Ln/Exp/Square served by ONE table
                                          set (natural_log_exp_and_others): no
                                          ACT_TABLE_LOAD thrash, no reciprocal.
  acc[j]   = sum_i src_i * kern[i,j]      PE matvec, PSUM-resident accumulator
                                          (row sum == col sum by symmetry)
  host: E[j] = 0.5*src_j*acc_j - 0.5*src_j^2*exp(-sigma*soft)/soft

Atoms in groups of GA=42 (3 coord rows per atom = 126 of 128 partitions);
3 groups form a 126-atom macro. All tiles padded to 128 partitions; padded
weight columns are zero, so padded q/kern rows compute to harmless zeros and
are masked by zero entries in the src weights.
"""
import numpy as np

GA = 42            # atoms per row group
ROWS = 128         # partitions per tile (3*GA = 126 used)
GPM = 3            # groups per macro block
MACRO = GA * GPM   # 126 atoms per macro
MAGIC = 12582912.0  # 1.5 * 2**23: (x + MAGIC) - MAGIC == round(x) for |x| < 2**22
NCORES = 8
CHUNK = 512        # PSUM bank limit for one matmul output (fp32 values)

_cache = {}
_pair_sq = None


def _get_pair_sq():
    """Register (once) a single-source custom DVE op computing out = in0^2.
    Used for the DVE share of the y->sq squares (PSUM source, so stock
    tensor_tensor(y, y) is illegal: both streams would be PSUM)."""
    global _pair_sq
    if _pair_sq is None:
        import concourse.dve_ops as dve_ops
        from concourse.dve_spec import Spec, Src0, sq, lower, _has_src1
        from concourse.dve_uop import DveOpSpec

        spec = Spec(body=sq(Src0),
                    reference=lambda in0, in1, s0, s1, imm2: in0 * in0)
        opcode = dve_ops._CUSTOM_DVE_ROW_BASE + len(dve_ops.OPS)
        shas = {}
        for ver in ("v3", "v4"):
            tmp = DveOpSpec(name="PAIR_SQ", opcode=opcode,
                            uops=lower(spec, ver=ver), rd1_en=_has_src1(spec))
            shas[ver] = tmp.sha(ver)
        op = dve_ops.DveOp("PAIR_SQ", spec, subdim=False, uops_sha=shas)
        dve_ops.OPS.append(op)
        dve_ops.CUSTOM_DVE_SPECS[op.name] = op.spec
        dve_ops._SUB_OPCODE_FOR_NAME[op.name] = opcode
        _pair_sq = op
    return _pair_sq


def _pin_lnexp_table(nc, mybir):
    """Bias the activation-table-load pass so Ln and Exp both resolve to the
    natural_log_exp_and_others set (the only set containing both): drop Ln
    from natural_log and Exp from exp_and_others in the (cached) table dict.
    Square stays available in every set, so the whole kernel needs ONE
    ACT_TABLE_LOAD instead of two per macro."""
    from concourse.hw_specs import get_activation_tables
    tabs = get_activation_tables(nc.m.arch)
    AF = mybir.ActivationFunctionType
    if "natural_log_exp_and_others" in tabs:
        tabs["natural_log"].discard(AF.Ln)
        tabs["exp_and_others"].discard(AF.Exp)


def _build(n_macros, n_groups, cols, sigma, soft):
    import concourse.bacc as bacc
    import concourse.mybir as mybir
    import concourse.tile as tile

    f32 = mybir.dt.float32
    f16 = mybir.dt.float16
    alu = mybir.AluOpType
    act = mybir.ActivationFunctionType
    pair_sq = _get_pair_sq()

    soft2 = float(np.float32(soft) * np.float32(soft))
    lnsig = float(np.log(np.float64(sigma)))
    chunks = [(c, min(cols, c + CHUNK)) for c in range(0, cols, CHUNK)]
    pw = -(-cols // CHUNK) * CHUNK

    nc = bacc.Bacc("TRN2", target_bir_lowering=False, debug=False)
    _pin_lnexp_table(nc, mybir)
    # const APs for activation biases, registered like the built-ins
    for name, val in (("c-soft2", soft2), ("c-lnsig", lnsig)):
        t = nc.alloc_sbuf_tensor(name, [128, 1], f32)
        nc.gpsimd.memset(t.ap(), val)
        nc.const_aps.aps[(f32, val)] = t.ap()
    nc.all_engine_barrier()

    FB = nc.declare_dram_parameter("FB", [ROWS, cols], f16, isOutput=False)
    NEGFA = nc.declare_dram_parameter("NEGFA", [ROWS, n_groups], f32, isOutput=False)
    CB = nc.declare_dram_parameter("CB", [ROWS, ROWS], f16, isOutput=False)
    CBN = nc.declare_dram_parameter("CBN", [ROWS, ROWS], f16, isOutput=False)
    ONESB = nc.declare_dram_parameter("ONESB", [ROWS, GPM * ROWS], f16, isOutput=False)
    SRCST = nc.declare_dram_parameter("SRCST", [ROWS, n_macros], f16, isOutput=False)
    OUT = nc.declare_dram_parameter("OUT", [1, cols], f32, isOutput=True)

    # group g -> (macro, slot); last macro may have < GPM groups
    def macro_groups(m):
        return [g for g in range(GPM * m, min(GPM * (m + 1), n_groups))]

    with tile.TileContext(nc) as tc:
        with tc.tile_pool(name="const", bufs=1) as cpool, \
             tc.tile_pool(name="work", bufs=3) as pool, \
             tc.tile_pool(name="ypsum", bufs=2, space="PSUM") as ypool, \
             tc.tile_pool(name="qpsum", bufs=1, space="PSUM") as qpool, \
             tc.tile_pool(name="apsum", bufs=1, space="PSUM") as apool:
            fb = cpool.tile([ROWS, cols], f16)
            negfa = cpool.tile([ROWS, n_groups], f32)
            cb = cpool.tile([ROWS, ROWS], f16)
            cbn = cpool.tile([ROWS, ROWS], f16)
            onesb = cpool.tile([ROWS, GPM * ROWS], f16)
            srcst = cpool.tile([ROWS, n_macros], f16)
            nc.sync.dma_start(fb[:], FB[:])
            nc.sync.dma_start(negfa[:], NEGFA[:])
            nc.sync.dma_start(cb[:], CB[:])
            nc.sync.dma_start(cbn[:], CBN[:])
            nc.sync.dma_start(onesb[:], ONESB[:])
            nc.sync.dma_start(srcst[:], SRCST[:])

            acc = apool.tile([1, pw], f32)

            sq_on_act = 0  # alternate the y->sq square between ACT and DVE
            for m in range(n_macros):
                groups = macro_groups(m)
                q = qpool.tile([ROWS, pw], f32, tag="q")
                for ti, g in enumerate(groups):
                    f = pool.tile([ROWS, cols], f16, tag="f")
                    nc.vector.tensor_scalar(f[:], fb[:], negfa[:, g:g + 1],
                                            None, alu.add)
                    r = pool.tile([ROWS, cols], f16, tag="r")
                    nc.vector.tensor_scalar(r[:], f[:], MAGIC, MAGIC,
                                            alu.add, alu.subtract)
                    y = ypool.tile([ROWS, pw], f32, tag="y")
                    for (c0, c1) in chunks:
                        nc.tensor.matmul(y[:, c0:c1], cb[:], f[:, c0:c1],
                                         start=True, stop=False)
                    for (c0, c1) in chunks:
                        nc.tensor.matmul(y[:, c0:c1], cbn[:], r[:, c0:c1],
                                         start=False, stop=True)
                    sq = pool.tile([ROWS, cols], f16, tag="sq")
                    if sq_on_act:
                        nc.scalar.activation(sq[:], y[:, 0:cols], act.Square)
                    else:
                        nc.vector._custom_dve(pair_sq, out=sq[:],
                                              in0=y[:, 0:cols])
                    sq_on_act ^= 1
                    ob = onesb[:, ROWS * ti:ROWS * (ti + 1)]
                    for (c0, c1) in chunks:
                        nc.tensor.matmul(q[:, c0:c1], ob, sq[:, c0:c1],
                                         start=(ti == 0),
                                         stop=(ti == len(groups) - 1))
                # tail: kern = exp(-0.5*t - sigma*exp(0.5*t + ln(sigma)))
                # (padded q rows are written as zeros by the zero ob columns,
                # so the tail is safe on all 128 rows; srcst zeros mask them)
                t = pool.tile([ROWS, cols], f16, tag="t")
                nc.scalar.activation(t[:], q[:, 0:cols], act.Ln, bias=soft2)
                rtp = pool.tile([ROWS, cols], f16, tag="rtp")
                nc.scalar.activation(rtp[:], t[:], act.Exp,
                                     bias=lnsig, scale=0.5)
                w = pool.tile([ROWS, cols], f16, tag="w")
                nc.vector.scalar_tensor_tensor(w[:], t[:], -0.5, rtp[:],
                                               alu.mult, alu.subtract)
                kern = pool.tile([ROWS, cols], f16, tag="kern")
                nc.scalar.activation(kern[:], w[:], act.Exp)
                for (c0, c1) in chunks:
                    nc.tensor.matmul(acc[0:1, c0:c1], srcst[:, m:m + 1],
                                     kern[:, c0:c1],
                                     start=(m == 0), stop=(m == n_macros - 1))

            eo = pool.tile([1, cols], f32, tag="eo")
            nc.scalar.copy(eo[:], acc[0:1, 0:cols])
            nc.sync.dma_start(OUT[:], eo[:])
    nc.compile()
    return nc


def _get_program(n_macros, n_groups, cols, sigma, soft):
    key = (n_macros, n_groups, cols, round(sigma, 9), round(soft, 9))
    if key not in _cache:
        _cache[key] = _build(n_macros, n_groups, cols, sigma, soft)
    return _cache[key]


LAST_EXEC_TIME_NS = None


def kernel(pos, batch, cell, source, screening, softening, *, _trace=False):
    global LAST_EXEC_TIME_NS
    from concourse.bass_utils import run_bass_kernel_spmd

    pos = np.asarray(pos)
    cell = np.asarray(cell)
    source = np.asarray(source, dtype=np.float32)
    sigma = float(np.asarray(screening, dtype=np.float32))
    soft = float(np.asarray(softening, dtype=np.float32))

    n = pos.shape[0]
    nb = cell.shape[0]
    bi = np.asarray(batch).astype(np.int64)
    counts = np.bincount(bi, minlength=nb)
    starts = np.concatenate([[0], np.cumsum(counts)])
    assert nb == NCORES and np.all(np.diff(bi) >= 0)

    # host precompute in float64; fs = frac - 0.5 halves the fp16 repr error
    inv = np.linalg.inv(cell.astype(np.float64))
    fs = np.empty((n, 3), dtype=np.float64)
    for g in range(nb):
        i0, i1 = starts[g], starts[g + 1]
        fs[i0:i1] = pos[i0:i1].astype(np.float64) @ inv[g] - 0.5
    fs16 = fs.astype(np.float16)

    namax = int(counts.max())
    cols = -(-namax // 8) * 8          # padded atom count per core
    n_groups = -(-namax // GA)
    n_macros = -(-namax // MACRO)
    diag_c = float(np.exp(-np.float64(sigma) * np.float64(soft)) / np.float64(soft))

    idx_atom = np.arange(3 * GA) // 3
    idx_k = np.arange(3 * GA) % 3

    in_maps = []
    for g in range(nb):
        i0, i1 = starts[g], starts[g + 1]
        ng = i1 - i0
        fpad = np.zeros((cols, 3), dtype=np.float16)
        fpad[:ng] = fs16[i0:i1]

        fb = np.zeros((ROWS, cols), dtype=np.float16)
        fb[:3 * GA] = np.tile(fpad.T, (GA, 1))
        negfa = np.zeros((ROWS, n_groups), dtype=np.float32)
        for t in range(n_groups):
            a = np.minimum(t * GA + idx_atom, cols - 1)
            negfa[:3 * GA, t] = -fpad[a, idx_k].astype(np.float32)
        C = cell[g].astype(np.float16)
        cbm = np.zeros((ROWS, ROWS), dtype=np.float16)
        for i in range(GA):
            cbm[3 * i:3 * i + 3, 3 * i:3 * i + 3] = C
        onesb = np.zeros((ROWS, GPM, ROWS), dtype=np.float16)
        for t in range(GPM):
            for i in range(GA):
                onesb[3 * i:3 * i + 3, t, GA * t + i] = 1.0
        onesb = np.ascontiguousarray(onesb.reshape(ROWS, GPM * ROWS))
        spad = np.zeros(MACRO * n_macros, dtype=np.float16)
        spad[:ng] = source[i0:i1].astype(np.float16)
        srcst = np.zeros((ROWS, n_macros), dtype=np.float16)
        srcst[:MACRO] = spad.reshape(n_macros, MACRO).T
        in_maps.append({
            "FB": fb, "NEGFA": negfa, "CB": cbm, "CBN": -cbm,
            "ONESB": onesb, "SRCST": srcst,
        })

    nc = _get_program(n_macros, n_groups, cols, sigma, soft)
    res = run_bass_kernel_spmd(nc, in_maps, list(range(NCORES)), trace=_trace)
    LAST_EXEC_TIME_NS = res.exec_time_ns

    out = np.zeros((n, 1), dtype=np.float32)
    for g in range(nb):
        i0, i1 = starts[g], starts[g + 1]
        ng = i1 - i0
        accg = res.results[g]["OUT"][0, :ng].astype(np.float64)
        s = source[i0:i1].astype(np.float64)
        out[i0:i1, 0] = (0.5 * s * accg - 0.5 * s * s * diag_c).astype(np.float32)
    return out


# revision 8
# speedup vs baseline: 1.4416x; 1.0526x over previous
"""Trainium2 Bass kernel: dense screened-Coulomb pair energy with periodic
minimum-image convention (N=6144 atoms, B=8 cells), row-summed per atom.

batch is sorted and cross-graph pairs are masked, so the N x N problem is
block-diagonal over the 8 graphs: one graph per NeuronCore.

Math (fractional coords, fs = frac - 0.5):
  f_k[i,j] = fs_k[j] - fs_k[i]            DVE tensor_scalar fp16
  r_k      = round(f_k)                   DVE magic-number round fp16
  y        = (f - r) @ C                  PE: 2 accumulating fp16 matmuls
                                          (block-diag cell stationary, 128-wide
                                          weights for fast-weight-load)
  sq       = y^2                          ACT Square / custom DVE op (split)
  q        = sum_k y_k^2                  PE: ones-blockdiag fp16 matmul
  kern     = exp(-sigma*r)/r, r=sqrt(q+soft^2)
           = exp(-0.5*t - sigma*exp(0.5*t + ln(sigma)))  with t = ln(q+soft^2)
                                          ACT Ln, ACT Exp, DVE stt, ACT Exp
                                          -- Ln/Exp/Square served by ONE table
                                          set (natural_log_exp_and_others): no
                                          ACT_TABLE_LOAD thrash, no reciprocal.
  acc[j]   = sum_i src_i * kern[i,j]      PE matvec, PSUM-resident accumulator
                                          (row sum == col sum by symmetry)
  host: E[j] = 0.5*src_j*acc_j - 0.5*src_j^2*exp(-sigma*soft)/soft

Atoms in groups of GA=42 (3 coord rows per atom = 126 of 128 partitions);
3 groups form a 126-atom macro. All tiles padded to 128 partitions; padded
weight columns are zero, so padded q/kern rows compute to harmless zeros and
are masked by zero entries in the src weights.
"""
import numpy as np

GA = 42            # atoms per row group
ROWS = 128         # partitions per tile (3*GA = 126 used)
GPM = 3            # groups per macro block
MACRO = GA * GPM   # 126 atoms per macro
MAGIC = 12582912.0  # 1.5 * 2**23: (x + MAGIC) - MAGIC == round(x) for |x| < 2**22
NCORES = 8
CHUNK = 512        # PSUM bank limit for one matmul output (fp32 values)

_cache = {}
_custom_ops = None


def _register_custom(name, spec):
    import concourse.dve_ops as dve_ops
    from concourse.dve_spec import lower, _has_src1
    from concourse.dve_uop import DveOpSpec

    opcode = dve_ops._CUSTOM_DVE_ROW_BASE + len(dve_ops.OPS)
    shas = {}
    for ver in ("v3", "v4"):
        tmp = DveOpSpec(name=name, opcode=opcode,
                        uops=lower(spec, ver=ver), rd1_en=_has_src1(spec))
        shas[ver] = tmp.sha(ver)
    op = dve_ops.DveOp(name, spec, subdim=False, uops_sha=shas)
    dve_ops.OPS.append(op)
    dve_ops.CUSTOM_DVE_SPECS[op.name] = op.spec
    dve_ops._SUB_OPCODE_FOR_NAME[op.name] = opcode
    return op


def _get_custom_ops():
    """Register (once) two custom DVE ops:
    PAIR_SQ: out = in0^2 (single source; PSUM-legal square for the DVE share
             of the y->sq work -- stock tensor_tensor(y, y) would need two
             PSUM streams, which the STT struct forbids).
    GWRAP:   out = f - round(f) with f = in0 + s0, round via the fp32
             magic-number trick. One op replaces the f and r tensor_scalars
             and halves the y matmuls (single stationary weight cb)."""
    global _custom_ops
    if _custom_ops is None:
        from concourse.dve_spec import Spec, Src0, C0, C1, sq

        sq_spec = Spec(body=sq(Src0),
                       reference=lambda in0, in1, s0, s1, imm2: in0 * in0)
        f = Src0 + C0
        g_spec = Spec(body=f - ((f + C1) - C1),
                      reference=lambda in0, in1, s0, s1, imm2:
                      (in0 + s0) - (((in0 + s0) + s1) - s1))
        _custom_ops = (_register_custom("PAIR_SQ", sq_spec),
                       _register_custom("GWRAP", g_spec))
    return _custom_ops


def _pin_lnexp_table(nc, mybir):
    """Bias the activation-table-load pass so Ln and Exp both resolve to the
    natural_log_exp_and_others set (the only set containing both): drop Ln
    from natural_log and Exp from exp_and_others in the (cached) table dict.
    Square stays available in every set, so the whole kernel needs ONE
    ACT_TABLE_LOAD instead of two per macro."""
    from concourse.hw_specs import get_activation_tables
    tabs = get_activation_tables(nc.m.arch)
    AF = mybir.ActivationFunctionType
    if "natural_log_exp_and_others" in tabs:
        tabs["natural_log"].discard(AF.Ln)
        tabs["exp_and_others"].discard(AF.Exp)


def _build(n_macros, n_groups, cols, sigma, soft):
    import concourse.bacc as bacc
    import concourse.mybir as mybir
    import concourse.tile as tile

    f32 = mybir.dt.float32
    f16 = mybir.dt.float16
    alu = mybir.AluOpType
    act = mybir.ActivationFunctionType
    pair_sq, gwrap = _get_custom_ops()

    soft2 = float(np.float32(soft) * np.float32(soft))
    lnsig = float(np.log(np.float64(sigma)))
    chunks = [(c, min(cols, c + CHUNK)) for c in range(0, cols, CHUNK)]
    pw = -(-cols // CHUNK) * CHUNK

    nc = bacc.Bacc("TRN2", target_bir_lowering=False, debug=False)
    _pin_lnexp_table(nc, mybir)
    # const APs for activation biases, registered like the built-ins
    for name, val in (("c-soft2", soft2), ("c-lnsig", lnsig)):
        t = nc.alloc_sbuf_tensor(name, [128, 1], f32)
        nc.gpsimd.memset(t.ap(), val)
        nc.const_aps.aps[(f32, val)] = t.ap()
    nc.all_engine_barrier()

    FB = nc.declare_dram_parameter("FB", [ROWS, cols], f16, isOutput=False)
    NEGFA = nc.declare_dram_parameter("NEGFA", [ROWS, n_groups], f32, isOutput=False)
    CB = nc.declare_dram_parameter("CB", [ROWS, ROWS], f16, isOutput=False)
    ONESB = nc.declare_dram_parameter("ONESB", [ROWS, GPM * ROWS], f16, isOutput=False)
    SRCST = nc.declare_dram_parameter("SRCST", [ROWS, n_macros], f16, isOutput=False)
    OUT = nc.declare_dram_parameter("OUT", [1, cols], f32, isOutput=True)

    # group g -> (macro, slot); last macro may have < GPM groups
    def macro_groups(m):
        return [g for g in range(GPM * m, min(GPM * (m + 1), n_groups))]

    with tile.TileContext(nc) as tc:
        with tc.tile_pool(name="const", bufs=1) as cpool, \
             tc.tile_pool(name="work", bufs=4) as pool, \
             tc.tile_pool(name="ypsum", bufs=2, space="PSUM") as ypool, \
             tc.tile_pool(name="qpsum", bufs=1, space="PSUM") as qpool, \
             tc.tile_pool(name="apsum", bufs=1, space="PSUM") as apool:
            fb = cpool.tile([ROWS, cols], f16)
            negfa = cpool.tile([ROWS, n_groups], f32)
            cb = cpool.tile([ROWS, ROWS], f16)
            onesb = cpool.tile([ROWS, GPM * ROWS], f16)
            srcst = cpool.tile([ROWS, n_macros], f16)
            nc.sync.dma_start(fb[:], FB[:])
            nc.gpsimd.dma_start(negfa[:], NEGFA[:])
            nc.scalar.dma_start(cb[:], CB[:])
            nc.gpsimd.dma_start(onesb[:], ONESB[:])
            nc.scalar.dma_start(srcst[:], SRCST[:])

            acc = apool.tile([1, pw], f32)

            sq_on_act = 0  # per-macro square pattern: [DVE, ACT, ACT]
            for m in range(n_macros):
                groups = macro_groups(m)
                q = qpool.tile([ROWS, pw], f32, tag="q")
                for ti, g in enumerate(groups):
                    gt = pool.tile([ROWS, cols], f16, tag="g")
                    nc.vector._custom_dve(gwrap, out=gt[:], in0=fb[:],
                                          s0=negfa[:, g:g + 1], s1=MAGIC)
                    y = ypool.tile([ROWS, pw], f32, tag="y")
                    for (c0, c1) in chunks:
                        nc.tensor.matmul(y[:, c0:c1], cb[:], gt[:, c0:c1],
                                         start=True, stop=True)
                    sq = pool.tile([ROWS, cols], f16, tag="sq")
                    if sq_on_act:
                        nc.scalar.activation(sq[:], y[:, 0:cols], act.Square)
                    else:
                        nc.vector._custom_dve(pair_sq, out=sq[:],
                                              in0=y[:, 0:cols])
                    sq_on_act = 0 if ti == len(groups) - 1 else 1
                    ob = onesb[:, ROWS * ti:ROWS * (ti + 1)]
                    for (c0, c1) in chunks:
                        nc.tensor.matmul(q[:, c0:c1], ob, sq[:, c0:c1],
                                         start=(ti == 0),
                                         stop=(ti == len(groups) - 1))
                # tail: kern = exp(-0.5*t - sigma*exp(0.5*t + ln(sigma)))
                # (padded q rows are written as zeros by the zero ob columns,
                # so the tail is safe on all 128 rows; srcst zeros mask them)
                t = pool.tile([ROWS, cols], f16, tag="t")
                nc.scalar.activation(t[:], q[:, 0:cols], act.Ln, bias=soft2)
                rtp = pool.tile([ROWS, cols], f16, tag="rtp")
                nc.scalar.activation(rtp[:], t[:], act.Exp,
                                     bias=lnsig, scale=0.5)
                w = pool.tile([ROWS, cols], f16, tag="w")
                nc.vector.scalar_tensor_tensor(w[:], t[:], -0.5, rtp[:],
                                               alu.mult, alu.subtract)
                kern = pool.tile([ROWS, cols], f16, tag="kern")
                nc.scalar.activation(kern[:], w[:], act.Exp)
                for (c0, c1) in chunks:
                    nc.tensor.matmul(acc[0:1, c0:c1], srcst[:, m:m + 1],
                                     kern[:, c0:c1],
                                     start=(m == 0), stop=(m == n_macros - 1))

            eo = pool.tile([1, cols], f32, tag="eo")
            nc.scalar.copy(eo[:], acc[0:1, 0:cols])
            nc.sync.dma_start(OUT[:], eo[:])
    nc.compile()
    return nc


def _get_program(n_macros, n_groups, cols, sigma, soft):
    key = (n_macros, n_groups, cols, round(sigma, 9), round(soft, 9))
    if key not in _cache:
        _cache[key] = _build(n_macros, n_groups, cols, sigma, soft)
    return _cache[key]


LAST_EXEC_TIME_NS = None


def kernel(pos, batch, cell, source, screening, softening, *, _trace=False):
    global LAST_EXEC_TIME_NS
    from concourse.bass_utils import run_bass_kernel_spmd

    pos = np.asarray(pos)
    cell = np.asarray(cell)
    source = np.asarray(source, dtype=np.float32)
    sigma = float(np.asarray(screening, dtype=np.float32))
    soft = float(np.asarray(softening, dtype=np.float32))

    n = pos.shape[0]
    nb = cell.shape[0]
    bi = np.asarray(batch).astype(np.int64)
    counts = np.bincount(bi, minlength=nb)
    starts = np.concatenate([[0], np.cumsum(counts)])
    assert nb == NCORES and np.all(np.diff(bi) >= 0)

    # host precompute in float64; fs = frac - 0.5 halves the fp16 repr error
    inv = np.linalg.inv(cell.astype(np.float64))
    fs = np.empty((n, 3), dtype=np.float64)
    for g in range(nb):
        i0, i1 = starts[g], starts[g + 1]
        fs[i0:i1] = pos[i0:i1].astype(np.float64) @ inv[g] - 0.5
    fs16 = fs.astype(np.float16)

    namax = int(counts.max())
    cols = -(-namax // 8) * 8          # padded atom count per core
    n_groups = -(-namax // GA)
    n_macros = -(-namax // MACRO)
    diag_c = float(np.exp(-np.float64(sigma) * np.float64(soft)) / np.float64(soft))

    idx_atom = np.arange(3 * GA) // 3
    idx_k = np.arange(3 * GA) % 3

    in_maps = []
    for g in range(nb):
        i0, i1 = starts[g], starts[g + 1]
        ng = i1 - i0
        fpad = np.zeros((cols, 3), dtype=np.float16)
        fpad[:ng] = fs16[i0:i1]

        fb = np.zeros((ROWS, cols), dtype=np.float16)
        fb[:3 * GA] = np.tile(fpad.T, (GA, 1))
        negfa = np.zeros((ROWS, n_groups), dtype=np.float32)
        for t in range(n_groups):
            a = np.minimum(t * GA + idx_atom, cols - 1)
            negfa[:3 * GA, t] = -fpad[a, idx_k].astype(np.float32)
        C = cell[g].astype(np.float16)
        cbm = np.zeros((ROWS, ROWS), dtype=np.float16)
        for i in range(GA):
            cbm[3 * i:3 * i + 3, 3 * i:3 * i + 3] = C
        onesb = np.zeros((ROWS, GPM, ROWS), dtype=np.float16)
        for t in range(GPM):
            for i in range(GA):
                onesb[3 * i:3 * i + 3, t, GA * t + i] = 1.0
        onesb = np.ascontiguousarray(onesb.reshape(ROWS, GPM * ROWS))
        spad = np.zeros(MACRO * n_macros, dtype=np.float16)
        spad[:ng] = source[i0:i1].astype(np.float16)
        srcst = np.zeros((ROWS, n_macros), dtype=np.float16)
        srcst[:MACRO] = spad.reshape(n_macros, MACRO).T
        in_maps.append({
            "FB": fb, "NEGFA": negfa, "CB": cbm,
            "ONESB": onesb, "SRCST": srcst,
        })

    nc = _get_program(n_macros, n_groups, cols, sigma, soft)
    res = run_bass_kernel_spmd(nc, in_maps, list(range(NCORES)), trace=_trace)
    LAST_EXEC_TIME_NS = res.exec_time_ns

    out = np.zeros((n, 1), dtype=np.float32)
    for g in range(nb):
        i0, i1 = starts[g], starts[g + 1]
        ng = i1 - i0
        accg = res.results[g]["OUT"][0, :ng].astype(np.float64)
        s = source[i0:i1].astype(np.float64)
        out[i0:i1, 0] = (0.5 * s * accg - 0.5 * s * s * diag_c).astype(np.float32)
    return out
